# revision 1
# baseline (speedup 1.0000x reference)
"""Distributed Trainium2 kernel for the MemoryEfficientCLIPLoss problem.

Strategy (SigLIP-style row sharding, 8 cores):
  - Core k owns rows [k*1024, (k+1)*1024) of both feature matrices.
  - Each core normalizes its own Y shard, transposes it to [D, rows] (PE
    transpose), and an AllGather distributes the normalized-transposed Y
    to every core (bf16, 1 MB per rank).
  - Each core computes its [1024, 8192] slab of E = exp((S - 1)/T) where
    S = Xn @ Yn.T (cosine similarities), accumulating:
      * row sums of E for its 1024 rows   (fused into the exp activation)
      * partial column sums of E (1 x 8192, ones-vector matmuls in PSUM,
        software-pipelined one step behind the exp to keep PE busy)
      * raw diagonal dots x_i . y_i and inverse norms (positive pair)
  - Host sums the 8 partial column-sum vectors, applies logs and the mean
    in float64, and returns the scalar loss.

Device compute is bf16 matmul (f32 PSUM accumulation) + f32 exp on the
scalar engine. X rows are fed to the matmul unnormalized; the 1/||x_i||
factor rides in the per-partition activation scale.
"""

import sys

sys.path.insert(0, "/opt/trn_rl_repo")

import numpy as np

B, D, NCORES = 8192, 512, 8
SHARD = B // NCORES  # 1024
P = 128
MT = SHARD // P  # 8 row tiles per core
KC = D // P  # 4 contraction chunks
GW = 1024  # column group width (2 PSUM banks)
NG = B // GW  # 8 column groups
TEMPERATURE = 0.07
LOG2E = 1.4426950408889634
INV_TEMP = LOG2E / TEMPERATURE  # base-2 exponent scale (reference units)
NAT = 1.0 / TEMPERATURE  # natural-log exponent scale used on device

_CACHE = {}


def _build(variant="ag"):
    """variants: ag / agloopR (real kernel, main repeated R times),
    agsim (no collective, TimelineSim), agprep (no main loop),
    agmmR (main = matmuls only), agnoonesR (main = matmuls+exp)."""
    if variant in _CACHE:
        return _CACHE[variant]
    import re

    m = re.match(r"(ag|f8)(simprep|simnoones|prep|mm|noones|loop|sim)?(\d*)$", variant)
    assert m, variant
    use_fp8 = m.group(1) == "f8"
    kind = m.group(2) or "loop"
    reps = int(m.group(3) or "1")
    do_collective = kind not in ("sim", "simprep", "simnoones")
    do_main = kind not in ("prep", "simprep")
    do_act = kind in ("noones", "simnoones", "loop", "sim")
    do_ones = kind in ("loop", "sim")

    import concourse.bacc as bacc
    import concourse.mybir as mybir
    import concourse.tile as tile
    from concourse.masks import make_identity

    f32 = mybir.dt.float32
    bf16 = mybir.dt.bfloat16
    f8 = mybir.dt.float8e4
    Alu = mybir.AluOpType
    Act = mybir.ActivationFunctionType
    Ax = mybir.AxisListType

    nc = bacc.Bacc("TRN2", target_bir_lowering=False, debug=False, num_devices=NCORES)

    x_in = nc.dram_tensor("x_shard", [SHARD, D], f32, kind="ExternalInput").ap()
    y_in = nc.dram_tensor("y_shard", [SHARD, D], f32, kind="ExternalInput").ap()
    rs_out = nc.dram_tensor("row_sums", [P, MT * (NG + 1)], f32, kind="ExternalOutput").ap()
    cs_out = nc.dram_tensor("col_sums", [1, (NG + 1) * GW], f32, kind="ExternalOutput").ap()
    dxy_out = nc.dram_tensor("dxy", [P, MT], f32, kind="ExternalOutput").ap()
    rx_out = nc.dram_tensor("rxn", [P, MT], f32, kind="ExternalOutput").ap()
    ry_out = nc.dram_tensor("ryn", [P, MT], f32, kind="ExternalOutput").ap()

    with tile.TileContext(nc) as tc:
        with (
            tc.tile_pool(name="const", bufs=1) as cpool,
            tc.tile_pool(name="load", bufs=1) as load,
            tc.tile_pool(name="prep", bufs=3) as prep,
            tc.tile_pool(name="small", bufs=2) as small,
            tc.tile_pool(name="feat", bufs=1) as feat,
            tc.tile_pool(name="epool", bufs=3) as epool,
            tc.tile_pool(name="acc", bufs=1) as acc,
            tc.tile_pool(name="psum_s", bufs=3, space="PSUM") as psum_s,
            tc.tile_pool(name="psum_c", bufs=1, space="PSUM") as psum_c,
            tc.tile_pool(name="dram", bufs=1, space="DRAM") as dram,
        ):
            ones = cpool.tile([P, 1], bf16, tag="ones")
            nc.vector.memset(ones, 1.0)
            nbias = cpool.tile([P, 1], f32, tag="nbias")
            nc.vector.memset(nbias, -NAT)
            mmdt = f8 if use_fp8 else bf16
            ident = cpool.tile([P, P], bf16, tag="ident")
            make_identity(nc, ident)

            # big feature tiles: layout [p, c*W + j] = chunk c, free j
            yt_all = feat.tile([P, KC * B], mmdt, tag="yt_all")
            xt_all = feat.tile([P, KC * SHARD], mmdt, tag="xt_all")
            ynt_stage = feat.tile([P, KC * SHARD], mmdt, tag="ynt_stage")

            rxT = acc.tile([P, MT], f32, tag="rxT")  # (1/||x||)/T
            # bf16 per-group column accumulators (DVE adds)
            dxy = acc.tile([P, MT], f32, tag="dxy")
            rs_parts = acc.tile([P, MT * (NG + 1)], f32, tag="rs_parts")
            colsum = acc.tile([1, (NG + 1) * GW], f32, tag="colsum")
            ss_all = acc.tile([P, 2 * MT], f32, tag="ss_all")
            rr_all = acc.tile([P, 2 * MT], f32, tag="rr_all")  # [:, :MT]=x

            ynt_shard_dram = dram.tile([D, SHARD], mmdt, tag="ynt_shard")
            ynt_full_dram = dram.tile(
                [NCORES * D, SHARD], mmdt, tag="ynt_full", addr_space="Shared"
            )

            # ---- load shards: Y first (feeds the AllGather), chunked so
            # the norm/transpose pipeline starts after the first chunk ----
            xall = load.tile([P, MT * D], f32, tag="xall")
            yall = load.tile([P, MT * D], f32, tag="yall")
            yv = yall.rearrange("p (t d) -> p t d", d=D)
            yin3 = y_in.rearrange("(t p) d -> p t d", p=P)
            for t in range(0, MT, 2):
                nc.sync.dma_start(out=yv[:, t : t + 2], in_=yin3[:, t : t + 2])
            nc.sync.dma_start(
                out=xall.rearrange("p (t d) -> p t d", d=D),
                in_=x_in.rearrange("(t p) d -> p t d", p=P),
            )

            def xs(t):
                return xall[:, t * D : (t + 1) * D]

            def ys(t):
                return yall[:, t * D : (t + 1) * D]

            def transpose_tile(src_bf, dst_all, t):
                tp = psum_s.tile([P, KC * P], bf16, tag="ps", name="tp")
                for c in range(KC):
                    nc.tensor.transpose(
                        tp[:, c * P : (c + 1) * P],
                        src_bf[:, c * P : (c + 1) * P],
                        ident,
                    )
                # strided copy: dst_all[p, c*W + t*128 + q] = tp[p, c*128 + q]
                dst = dst_all.rearrange("p (c w) -> p c w", c=KC)[
                    :, :, t * P : (t + 1) * P
                ]
                nc.vector.tensor_copy(dst, tp.rearrange("p (c q) -> p c q", c=KC))

            # ---- Y path first: per-pair norms -> normalize -> transpose ----
            for t in range(0, MT, 2):
                for u in (t, t + 1):
                    scy = prep.tile([P, D], f32, tag="sc")
                    nc.vector.scalar_tensor_tensor(
                        out=scy, in0=ys(u), scalar=1.0, in1=ys(u),
                        op0=Alu.mult, op1=Alu.mult,
                        accum_out=ss_all[:, MT + u : MT + u + 1],
                    )
                rec_y = small.tile([P, 2], f32, tag="rec_y", name=f"rec_y{t}")
                nc.vector.reciprocal(rec_y, ss_all[:, MT + t : MT + t + 2])
                nc.scalar.sqrt(rr_all[:, MT + t : MT + t + 2], rec_y)
                for u in (t, t + 1):
                    ybf = prep.tile([P, D], bf16, tag="ybf")
                    nc.vector.tensor_scalar_mul(
                        ybf, ys(u), rr_all[:, MT + u : MT + u + 1]
                    )
                    transpose_tile(ybf, ynt_stage, u)

            # ---- AllGather the normalized-transposed Y shard ----
            nc.sync.dma_start(
                out=ynt_shard_dram.rearrange("(c p) j -> p c j", p=P),
                in_=ynt_stage.rearrange("p (c j) -> p c j", c=KC),
            )
            if do_collective:
                nc.gpsimd.collective_compute(
                    "AllGather",
                    Alu.bypass,
                    ins=[ynt_shard_dram.opt()],
                    outs=[ynt_full_dram.opt()],
                    replica_groups=[list(range(NCORES))],
                )
            else:
                # TimelineSim variant: pretend rank 0 only
                nc.sync.dma_start(
                    out=ynt_full_dram[0:D, :].rearrange("(c p) j -> p c j", p=P),
                    in_=ynt_stage.rearrange("p (c j) -> p c j", c=KC),
                )

            # ---- X path (overlaps the AllGather) ----
            for t in range(MT):
                scx = prep.tile([P, D], f32, tag="sc")
                nc.vector.scalar_tensor_tensor(
                    out=scx, in0=xs(t), scalar=1.0, in1=xs(t),
                    op0=Alu.mult, op1=Alu.mult,
                    accum_out=ss_all[:, t : t + 1],
                )
            rec_x = small.tile([P, MT], f32, tag="rec_x")
            nc.vector.reciprocal(rec_x, ss_all[:, 0:MT])
            nc.scalar.sqrt(rr_all[:, 0:MT], rec_x)
            nc.vector.tensor_scalar_mul(rxT, rr_all[:, 0:MT], NAT)
            for t in range(MT):
                xbf = prep.tile([P, D], bf16, tag="xbf")
                nc.vector.tensor_copy(xbf, xs(t))
                transpose_tile(xbf, xt_all, t)

            # ---- diagonal dots (raw) ----
            for t in range(MT):
                scd = prep.tile([P, D], f32, tag="sc")
                nc.vector.scalar_tensor_tensor(
                    out=scd, in0=xs(t), scalar=1.0, in1=ys(t),
                    op0=Alu.mult, op1=Alu.mult,
                    accum_out=dxy[:, t : t + 1],
                )

            # load gathered Y r-major so the main loop can start at r=0
            ytv = yt_all.rearrange("p (c r j) -> p c r j", c=KC, j=SHARD)
            if do_collective:
                for r in range(NCORES):
                    blk = ynt_full_dram[r * D : (r + 1) * D, :]
                    nc.sync.dma_start(
                        out=ytv[:, :, r, :],
                        in_=blk.rearrange("(c p) j -> p c j", p=P),
                    )
            else:
                nc.gpsimd.memset(yt_all, 0.0)
                nc.sync.dma_start(
                    out=ytv[:, :, 0, :],
                    in_=ynt_full_dram[0:D, :].rearrange("(c p) j -> p c j", p=P),
                )

            # ---- main: S tiles -> exp -> row sums (fused) + col sums ----
            state = {"pending": None}
            pcs = {}

            def emit_ones(e, g, m):
                # ones-matmuls accumulate column sums in PSUM, one m behind
                if g not in pcs:
                    pcs[g] = psum_c.tile([1, GW], f32, tag="pc", name=f"pc{g}")
                pc = pcs[g]
                for h in range(GW // 512):
                    nc.tensor.matmul(
                        pc[0:1, h * 512 : (h + 1) * 512],
                        ones,
                        e[:, h * 512 : (h + 1) * 512],
                        start=(m == 0),
                        stop=(m == MT - 1),
                    )
                if m == MT - 1:
                    nc.vector.tensor_copy(colsum[0:1, g * GW : (g + 1) * GW], pc)
                    del pcs[g]

            def main_section():
                for g in range(NG + 1):
                    xt3 = xt_all.rearrange("p (c j) -> p c j", c=KC)
                    if g == 0:
                        # own block, straight from the local staged transpose
                        src3 = ynt_stage.rearrange("p (c j) -> p c j", c=KC)
                        colbase = 0
                    else:
                        src3 = yt_all.rearrange("p (c j) -> p c j", c=KC)
                        colbase = (g - 1) * GW
                    for m in range(MT):
                        ps = psum_s.tile([P, GW], f32, tag="ps", name="ps")
                        if use_fp8:
                            for cc in range(0, KC, 2):
                                lhs3 = xt3[:, cc : cc + 2, m * P : (m + 1) * P]
                                for h in range(GW // 512):
                                    col0 = colbase + h * 512
                                    nc.tensor.matmul(
                                        ps[:, h * 512 : (h + 1) * 512],
                                        lhs3,
                                        src3[:, cc : cc + 2, col0 : col0 + 512],
                                        start=(cc == 0),
                                        stop=(cc == KC - 2),
                                        perf_mode=mybir.MatmulPerfMode.DoubleRow,
                                    )
                        else:
                            for c in range(KC):
                                lhs = xt_all[
                                    :, c * SHARD + m * P : c * SHARD + (m + 1) * P
                                ]
                                for h in range(GW // 512):
                                    col0 = colbase + h * 512
                                    nc.tensor.matmul(
                                        ps[:, h * 512 : (h + 1) * 512],
                                        lhs,
                                        src3[:, c, col0 : col0 + 512],
                                        start=(c == 0),
                                        stop=(c == KC - 1),
                                    )
                        if state["pending"] is not None and do_ones:
                            emit_ones(*state["pending"])
                            state["pending"] = None
                        if not do_act:
                            junk = epool.tile([P, GW], bf16, tag="e", name="e")
                            nc.vector.tensor_copy(junk, ps)
                            continue
                        e = epool.tile([P, GW], bf16, tag="e", name="e")
                        j = m * (NG + 1) + g
                        nc.scalar.activation(
                            e, ps, Act.Exp,
                            bias=nbias,
                            scale=rxT[:, m : m + 1],
                            accum_out=rs_parts[:, j : j + 1],
                        )
                        if do_ones:
                            state["pending"] = (e, g, m)
                if do_ones and state["pending"] is not None:
                    emit_ones(*state["pending"])
                state["pending"] = None

            if not (do_act and do_main):
                nc.vector.memset(rs_parts, 1.0)
            if not (do_ones and do_main):
                nc.vector.memset(colsum, 1.0)
            if do_main:
                if reps > 1:
                    with tc.For_i(0, reps, 1):
                        main_section()
                else:
                    main_section()

            nc.sync.dma_start(rs_out, rs_parts)
            nc.sync.dma_start(cs_out, colsum)
            nc.sync.dma_start(dxy_out, dxy)
            nc.sync.dma_start(rx_out, rr_all[:, 0:MT])
            nc.sync.dma_start(ry_out, rr_all[:, MT : 2 * MT])

    nc.compile()
    _CACHE[variant] = nc
    return nc


def _make_in_maps(image_features, text_features):
    X = np.ascontiguousarray(np.asarray(image_features, dtype=np.float32))
    Y = np.ascontiguousarray(np.asarray(text_features, dtype=np.float32))
    assert X.shape == (B, D) and Y.shape == (B, D)
    in_maps = []
    for k in range(NCORES):
        sl = slice(k * SHARD, (k + 1) * SHARD)
        in_maps.append(
            {
                "x_shard": np.ascontiguousarray(X[sl]),
                "y_shard": np.ascontiguousarray(Y[sl]),
            }
        )
    return in_maps


def _assemble(results):
    """Combine per-core partials into the scalar loss (float64 epilogue).

    Device layout: 9 column-block "slots" per core; slot 0 is the core's own
    block (computed from the local staged transpose before the AllGather
    landed), slots 1..8 are AllGather blocks 0..7.  Core k's slot k+1
    duplicates its slot 0 and is discarded.
    """
    NS = NG + 1

    def flat(name):
        # device layout [128, MT] with row index i = m*128 + p  ->  [SHARD]
        return np.concatenate(
            [np.asarray(r[name], dtype=np.float64).T.reshape(-1) for r in results]
        )

    dxy = flat("dxy")
    rx = flat("rxn")
    ry = flat("ryn")

    row_sum = np.empty(B, dtype=np.float64)
    col_sum = np.zeros(B, dtype=np.float64)
    for k, r in enumerate(results):
        rs = np.asarray(r["row_sums"], dtype=np.float64)  # [128, MT*NS]
        rs = rs.reshape(P, MT, NS)
        keep = [s for s in range(NS) if s != k + 1]
        rs_k = rs[:, :, keep].sum(axis=2)  # [128, MT]
        row_sum[k * SHARD : (k + 1) * SHARD] = rs_k.T.reshape(-1)
        cs = np.asarray(r["col_sums"], dtype=np.float64).reshape(NS, GW)
        col_sum[k * SHARD : (k + 1) * SHARD] += cs[0]
        for s in range(1, NS):
            if s == k + 1:
                continue
            blk = s - 1
            col_sum[blk * GW : (blk + 1) * GW] += cs[s]
    sv = dxy * rx * ry * INV_TEMP - INV_TEMP
    logits = 2.0 * sv - np.log(row_sum) - np.log(col_sum)
    loss = -logits.mean() / 2.0
    return np.asarray(loss, dtype=np.float32)


def run_spmd(image_features, text_features, trace=False):
    from concourse.bass_utils import run_bass_kernel_spmd

    nc = _build("f8")
    in_maps = _make_in_maps(image_features, text_features)
    res = run_bass_kernel_spmd(nc, in_maps, list(range(NCORES)), trace=trace)
    return res


def kernel(image_features, text_features):
    res = run_spmd(image_features, text_features, trace=False)
    return _assemble(res.results)


if __name__ == "__main__":
    rng = np.random.default_rng(0)
    x = rng.standard_normal((B, D), dtype=np.float32)
    y = rng.standard_normal((B, D), dtype=np.float32)
    out = kernel(x, y)
    print("loss:", out)



# revision 3
# speedup vs baseline: 3.9030x; 3.9030x over previous
"""Distributed Trainium2 kernel for the MemoryEfficientCLIPLoss problem.

Strategy (SigLIP-style row sharding, 8 cores):
  - Core k owns rows [k*1024, (k+1)*1024) of both feature matrices.
  - Each core normalizes its own Y shard, transposes it to [D, rows] (PE
    transpose), converts to fp8, and an AllGather distributes the
    normalized-transposed Y to every core (0.5 MB per rank).
  - Each core computes its [1024, 8192] slab of E = exp((S - 1)/T) where
    S = Xn @ Yn.T (cosine similarities), in 4 column groups of 2048:
      * fp8 DoubleRow matmuls accumulate S in PSUM (f32)
      * one exp activation per [128, 2048] tile, row sums fused into the
        activation's accumulator output
      * column sums: DVE accumulates E tiles (bf16) over the 8 row tiles
        into a per-group [128, 2048] accumulator; a ones-vector matmul
        (software-pipelined one group behind) reduces across partitions
  - Host sums the 8 partial column-sum vectors, applies logs and the mean
    in float64, and returns the scalar loss.

Engine budget per core in the main loop (cost model): Activation ~66 us
(bound), DVE ~39 us, PE ~31 us.  X rows are fed to the matmul
unnormalized; the 1/||x_i|| factor rides in the per-partition activation
scale.  Norm square-sums run on the otherwise-idle Activation engine
during prep.
"""

import sys

sys.path.insert(0, "/opt/trn_rl_repo")

import numpy as np

B, D, NCORES = 8192, 512, 8
SHARD = B // NCORES  # 1024
P = 128
MT = SHARD // P  # 8 row tiles per core
KC = D // P  # 4 contraction chunks
GW = 2048  # column group width (4 PSUM banks)
NG = B // GW  # 4 column groups
TEMPERATURE = 0.07
LOG2E = 1.4426950408889634
INV_TEMP = LOG2E / TEMPERATURE  # base-2 exponent scale (reference units)
NAT = 1.0 / TEMPERATURE  # natural-log exponent scale used on device

_CACHE = {}


def _build(variant="f8"):
    """variants: f8 / f8loopR (real kernel, main repeated R times),
    f8sim (no collective, TimelineSim), f8prep (no main loop),
    f8mmR (main = matmuls only), f8noonesR (main = matmuls+exp)."""
    if variant in _CACHE:
        return _CACHE[variant]
    import re

    m = re.match(r"(f8|ag)(simprep|simnoones|prep|mm|noones|loop|sim)?(\d*)$", variant)
    assert m, variant
    use_fp8 = m.group(1) == "f8"
    kind = m.group(2) or "loop"
    reps = int(m.group(3) or "1")
    do_collective = kind not in ("sim", "simprep", "simnoones")
    do_main = kind not in ("prep", "simprep")
    do_act = kind in ("noones", "simnoones", "loop", "sim")
    do_ones = kind in ("loop", "sim")

    import concourse.bacc as bacc
    import concourse.mybir as mybir
    import concourse.tile as tile
    from concourse.masks import make_identity

    f32 = mybir.dt.float32
    bf16 = mybir.dt.bfloat16
    f8 = mybir.dt.float8e4
    Alu = mybir.AluOpType
    Act = mybir.ActivationFunctionType

    nc = bacc.Bacc("TRN2", target_bir_lowering=False, debug=False, num_devices=NCORES)

    x_in = nc.dram_tensor("x_shard", [SHARD, D], f32, kind="ExternalInput").ap()
    y_in = nc.dram_tensor("y_shard", [SHARD, D], f32, kind="ExternalInput").ap()
    rs_out = nc.dram_tensor("row_sums", [P, MT * NG], f32, kind="ExternalOutput").ap()
    cs_out = nc.dram_tensor("col_sums", [1, B], f32, kind="ExternalOutput").ap()
    dxy_out = nc.dram_tensor("dxy", [P, MT], f32, kind="ExternalOutput").ap()
    rx_out = nc.dram_tensor("rxn", [P, MT], f32, kind="ExternalOutput").ap()
    ry_out = nc.dram_tensor("ryn", [P, MT], f32, kind="ExternalOutput").ap()

    with tile.TileContext(nc) as tc:
        with (
            tc.tile_pool(name="const", bufs=1) as cpool,
            tc.tile_pool(name="load", bufs=1) as load,
            tc.tile_pool(name="prep", bufs=3) as prep,
            tc.tile_pool(name="small", bufs=2) as small,
            tc.tile_pool(name="feat", bufs=1) as feat,
            tc.tile_pool(name="epool", bufs=3) as epool,
            tc.tile_pool(name="acc", bufs=1) as acc,
            tc.tile_pool(name="psum_s", bufs=2, space="PSUM") as psum_s,
            tc.tile_pool(name="dram", bufs=1, space="DRAM") as dram,
        ):
            ones = cpool.tile([P, 1], bf16, tag="ones")
            nc.vector.memset(ones, 1.0)
            nbias = cpool.tile([P, 1], f32, tag="nbias")
            nc.vector.memset(nbias, -NAT)
            mmdt = f8 if use_fp8 else bf16
            ident = cpool.tile([P, P], bf16, tag="ident")
            make_identity(nc, ident)

            # big feature tiles: layout [p, c*W + j] = chunk c, free j
            yt_all = feat.tile([P, KC * B], mmdt, tag="yt_all")
            xt_all = feat.tile([P, KC * SHARD], mmdt, tag="xt_all")
            ynt_stage = feat.tile([P, KC * SHARD], mmdt, tag="ynt_stage")

            rxT = acc.tile([P, MT], f32, tag="rxT")  # (1/||x||)/T
            dxy = acc.tile([P, MT], f32, tag="dxy")
            rs_parts = acc.tile([P, MT * NG], f32, tag="rs_parts")
            colsum = acc.tile([1, B], f32, tag="colsum")
            cacc = acc.tile([P, B], bf16, tag="cacc")  # col partial sums
            ss_all = acc.tile([P, 2 * MT], f32, tag="ss_all")
            rr_all = acc.tile([P, 2 * MT], f32, tag="rr_all")  # [:, :MT]=x

            ynt_shard_dram = dram.tile([D, SHARD], mmdt, tag="ynt_shard")
            ynt_full_dram = dram.tile(
                [NCORES * D, SHARD], mmdt, tag="ynt_full", addr_space="Shared"
            )

            # ---- load shards: Y first (feeds the AllGather), chunked so
            # the norm/transpose pipeline starts after the first chunk ----
            xall = load.tile([P, MT * D], f32, tag="xall")
            yall = load.tile([P, MT * D], f32, tag="yall")
            yv = yall.rearrange("p (t d) -> p t d", d=D)
            yin3 = y_in.rearrange("(t p) d -> p t d", p=P)
            for t in range(0, MT, 2):
                nc.sync.dma_start(out=yv[:, t : t + 2], in_=yin3[:, t : t + 2])
            nc.sync.dma_start(
                out=xall.rearrange("p (t d) -> p t d", d=D),
                in_=x_in.rearrange("(t p) d -> p t d", p=P),
            )

            def xs(t):
                return xall[:, t * D : (t + 1) * D]

            def ys(t):
                return yall[:, t * D : (t + 1) * D]

            def transpose_tile(src_bf, dst_all, t):
                tp = psum_s.tile([P, KC * P], bf16, tag="ps", name="tp")
                for c in range(KC):
                    nc.tensor.transpose(
                        tp[:, c * P : (c + 1) * P],
                        src_bf[:, c * P : (c + 1) * P],
                        ident,
                    )
                # strided copy: dst_all[p, c*W + t*128 + q] = tp[p, c*128 + q]
                dst = dst_all.rearrange("p (c w) -> p c w", c=KC)[
                    :, :, t * P : (t + 1) * P
                ]
                nc.vector.tensor_copy(dst, tp.rearrange("p (c q) -> p c q", c=KC))

            # ---- Y path first: per-pair norms (Act engine) -> normalize
            # (DVE) -> transpose (PE) ----
            for t in range(0, MT, 2):
                for u in (t, t + 1):
                    scy = prep.tile([P, D], bf16, tag="sc")
                    nc.scalar.activation(
                        scy, ys(u), Act.Square,
                        accum_out=ss_all[:, MT + u : MT + u + 1],
                    )
                rec_y = small.tile([P, 2], f32, tag="rec_y", name=f"rec_y{t}")
                nc.vector.reciprocal(rec_y, ss_all[:, MT + t : MT + t + 2])
                nc.scalar.sqrt(rr_all[:, MT + t : MT + t + 2], rec_y)
                for u in (t, t + 1):
                    ybf = prep.tile([P, D], bf16, tag="ybf")
                    nc.vector.tensor_scalar_mul(
                        ybf, ys(u), rr_all[:, MT + u : MT + u + 1]
                    )
                    transpose_tile(ybf, ynt_stage, u)

            # ---- AllGather the normalized-transposed Y shard (fp8) ----
            nc.sync.dma_start(
                out=ynt_shard_dram.rearrange("(c p) j -> p c j", p=P),
                in_=ynt_stage.rearrange("p (c j) -> p c j", c=KC),
            )
            if do_collective:
                nc.gpsimd.collective_compute(
                    "AllGather",
                    Alu.bypass,
                    ins=[ynt_shard_dram.opt()],
                    outs=[ynt_full_dram.opt()],
                    replica_groups=[list(range(NCORES))],
                )

            # ---- X path (overlaps the AllGather) ----
            for t in range(MT):
                scx = prep.tile([P, D], bf16, tag="sc")
                nc.scalar.activation(
                    scx, xs(t), Act.Square, accum_out=ss_all[:, t : t + 1]
                )
            rec_x = small.tile([P, MT], f32, tag="rec_x")
            nc.vector.reciprocal(rec_x, ss_all[:, 0:MT])
            nc.scalar.sqrt(rr_all[:, 0:MT], rec_x)
            nc.vector.tensor_scalar_mul(rxT, rr_all[:, 0:MT], NAT)
            for t in range(MT):
                xbf = prep.tile([P, D], bf16, tag="xbf")
                nc.vector.tensor_copy(xbf, xs(t))
                transpose_tile(xbf, xt_all, t)

            # ---- diagonal dots (raw) ----
            for t in range(MT):
                scd = prep.tile([P, D], f32, tag="scd")
                nc.vector.scalar_tensor_tensor(
                    out=scd, in0=xs(t), scalar=1.0, in1=ys(t),
                    op0=Alu.mult, op1=Alu.mult,
                    accum_out=dxy[:, t : t + 1],
                )

            # load gathered Y (fp8) r-major so the main loop starts at r=0
            ytv = yt_all.rearrange("p (c r j) -> p c r j", c=KC, j=SHARD)
            if do_collective:
                for r in range(NCORES):
                    blk = ynt_full_dram[r * D : (r + 1) * D, :]
                    nc.sync.dma_start(
                        out=ytv[:, :, r, :],
                        in_=blk.rearrange("(c p) j -> p c j", p=P),
                    )
            else:
                # TimelineSim variant: fill own block only (timing-equivalent)
                nc.gpsimd.memset(yt_all, 0.0)
                nc.sync.dma_start(
                    out=ytv[:, :, 0, :],
                    in_=ynt_stage.rearrange("p (c j) -> p c j", c=KC),
                )

            # ---- main: S tiles -> exp -> row sums (fused) + col sums ----
            state = {"pending": None}

            def emit_ones(g):
                # reduce cacc[:, group g] across partitions into PSUM, then
                # copy to the colsum sbuf tile (DVE)
                pc = psum_s.tile([1, GW], f32, tag="ps", name=f"pc{g}")
                for h in range(GW // 512):
                    nc.tensor.matmul(
                        pc[0:1, h * 512 : (h + 1) * 512],
                        ones,
                        cacc[:, g * GW + h * 512 : g * GW + (h + 1) * 512],
                        start=True,
                        stop=True,
                    )
                nc.vector.tensor_copy(colsum[0:1, g * GW : (g + 1) * GW], pc)

            def main_section():
                src3 = yt_all.rearrange("p (c j) -> p c j", c=KC)
                for g in range(NG):
                    for m in range(MT):
                        ps = psum_s.tile([P, GW], f32, tag="ps", name="ps")
                        if use_fp8:
                            for cc in range(0, KC, 2):
                                lhs3 = xt_all.rearrange(
                                    "p (c j) -> p c j", c=KC
                                )[:, cc : cc + 2, m * P : (m + 1) * P]
                                for h in range(GW // 512):
                                    col0 = g * GW + h * 512
                                    nc.tensor.matmul(
                                        ps[:, h * 512 : (h + 1) * 512],
                                        lhs3,
                                        src3[:, cc : cc + 2, col0 : col0 + 512],
                                        start=(cc == 0),
                                        stop=(cc == KC - 2),
                                        perf_mode=mybir.MatmulPerfMode.DoubleRow,
                                    )
                        else:
                            for c in range(KC):
                                lhs = xt_all[
                                    :, c * SHARD + m * P : c * SHARD + (m + 1) * P
                                ]
                                for h in range(GW // 512):
                                    col0 = g * GW + h * 512
                                    nc.tensor.matmul(
                                        ps[:, h * 512 : (h + 1) * 512],
                                        lhs,
                                        src3[:, c, col0 : col0 + 512],
                                        start=(c == 0),
                                        stop=(c == KC - 1),
                                    )
                        # software-pipelined colsum reduce, one group behind
                        if m == 2 and state["pending"] is not None and do_ones:
                            emit_ones(state["pending"])
                            state["pending"] = None
                        if not do_act:
                            continue
                        gsl = slice(g * GW, (g + 1) * GW)
                        if m == 0:
                            # first row tile: activation writes the col
                            # accumulator directly (saves a DVE copy)
                            nc.scalar.activation(
                                cacc[:, gsl], ps, Act.Exp,
                                bias=nbias,
                                scale=rxT[:, m : m + 1],
                                accum_out=rs_parts[:, g : g + 1],
                            )
                        else:
                            e = epool.tile([P, GW], bf16, tag="e", name="e")
                            j = m * NG + g
                            nc.scalar.activation(
                                e, ps, Act.Exp,
                                bias=nbias,
                                scale=rxT[:, m : m + 1],
                                accum_out=rs_parts[:, j : j + 1],
                            )
                            if do_ones:
                                nc.vector.tensor_tensor(
                                    out=cacc[:, gsl], in0=cacc[:, gsl],
                                    in1=e, op=Alu.add,
                                )
                    if do_ones:
                        state["pending"] = g
                if do_ones and state["pending"] is not None:
                    emit_ones(state["pending"])
                state["pending"] = None

            if not (do_act and do_main):
                nc.vector.memset(rs_parts, 1.0)
            if not (do_ones and do_main):
                nc.vector.memset(colsum, 1.0)
            if do_main:
                if reps > 1:
                    with tc.For_i(0, reps, 1):
                        main_section()
                else:
                    main_section()

            nc.sync.dma_start(rs_out, rs_parts)
            nc.sync.dma_start(cs_out, colsum)
            nc.sync.dma_start(dxy_out, dxy)
            nc.sync.dma_start(rx_out, rr_all[:, 0:MT])
            nc.sync.dma_start(ry_out, rr_all[:, MT : 2 * MT])

    nc.compile()
    _CACHE[variant] = nc
    return nc


def _make_in_maps(image_features, text_features):
    X = np.ascontiguousarray(np.asarray(image_features, dtype=np.float32))
    Y = np.ascontiguousarray(np.asarray(text_features, dtype=np.float32))
    assert X.shape == (B, D) and Y.shape == (B, D)
    in_maps = []
    for k in range(NCORES):
        sl = slice(k * SHARD, (k + 1) * SHARD)
        in_maps.append(
            {
                "x_shard": np.ascontiguousarray(X[sl]),
                "y_shard": np.ascontiguousarray(Y[sl]),
            }
        )
    return in_maps


def _assemble(results):
    """Combine per-core partials into the scalar loss (float64 epilogue).

    Device layout: rs_parts[p, m*NG + g] holds the row sum of row
    i = m*128 + p (within the core's shard) over column group g; col_sums
    is the core's partial column-sum vector over all 8192 columns.
    """

    def flat(name):
        # device layout [128, MT] with row index i = m*128 + p  ->  [SHARD]
        return np.concatenate(
            [np.asarray(r[name], dtype=np.float64).T.reshape(-1) for r in results]
        )

    dxy = flat("dxy")
    rx = flat("rxn")
    ry = flat("ryn")

    row_sum = np.empty(B, dtype=np.float64)
    col_sum = np.zeros(B, dtype=np.float64)
    for k, r in enumerate(results):
        rs = np.asarray(r["row_sums"], dtype=np.float64)  # [128, MT*NG]
        rs_k = rs.reshape(P, MT, NG).sum(axis=2)  # [128, MT]
        row_sum[k * SHARD : (k + 1) * SHARD] = rs_k.T.reshape(-1)
        col_sum += np.asarray(r["col_sums"], dtype=np.float64).reshape(B)
    sv = dxy * rx * ry * INV_TEMP - INV_TEMP
    logits = 2.0 * sv - np.log(row_sum) - np.log(col_sum)
    loss = -logits.mean() / 2.0
    return np.asarray(loss, dtype=np.float32)


def run_spmd(image_features, text_features, trace=False):
    from concourse.bass_utils import run_bass_kernel_spmd

    nc = _build("f8")
    in_maps = _make_in_maps(image_features, text_features)
    res = run_bass_kernel_spmd(nc, in_maps, list(range(NCORES)), trace=trace)
    return res


def kernel(image_features, text_features):
    res = run_spmd(image_features, text_features, trace=False)
    return _assemble(res.results)


if __name__ == "__main__":
    rng = np.random.default_rng(0)
    x = rng.standard_normal((B, D), dtype=np.float32)
    y = rng.standard_normal((B, D), dtype=np.float32)
    out = kernel(x, y)
    print("loss:", out)
